# revision 26
# baseline (speedup 1.0000x reference)
"""Trainium2 Bass kernel for BodyConvClothGraphConvolution.

Reference computation (R = C = 8192, D = H = 256):
    X0  = notes @ w                     # (R+C, H)
    top = X0[:R] + weight @ X0[R:]      # (R, H)
    out = concat([relu(top + b), relu(b)*ones(C,H), X0[R:]], axis=0)

Key algebraic restructure vs the v0 kernel: weight @ (Nb @ w) is computed as
(weight @ Nb) @ w.  The big contraction runs against the *raw* body notes,
so the (R+C, D) -> H projection no longer has to be replicated on every
core (v0 spent ~14 us/core of PE redundantly projecting all 8192 body
rows).  Per-core PE drops to ~60 us; DMA (~22 MB -> ~62 us) and PE are now
balanced at the ridge.

Sharding (8 cores, zero cross-core communication):
  - weight rows (and cloth-note rows, and the x0b output rows) sharded 8-way.
  - body notes nbc ([c,d]-blocked, 4.2 MB) replicated as the stationary
    operand of the big matmul; w/b replicated.

Per-core kernel (bf16 matmul inputs, fp32 PSUM accum, bf16 outputs):
  psY[d,m]      = NcT_shard (identity matmul)    (cloth notes fold into the
                                                  [d,m] accumulators exactly:
                                                  top = (Ncl + W@Nb) @ w)
  x0bT[h,c_own] = w^T @ NbT_own                  (psT banks, early)
  psY[d,m]     += nbc[c,d-blk]^T @ Wslab[c,m]    (streamed, 64 c-blocks x
                                                  16 slabs, 256 LDW+MM pairs)
  psT[h,m]      = w^T @ psY ; relu(+b) -> out    (tail, mc-staggered)
DMA rings: SP ring streams the 16.8 MB weight (16 slabs of 1.05 MB);
ACT ring carries notes/w/b loads (5.4 MB) and all output stores.

Measured: ~89 us/iter loop-slope (v0 baseline measures ~102 by the same
method; its graded single-shot was 100866 ns).  PE-bound: the toolchain
emits one LDWEIGHTS per matmul (no dedup), so each N=512 pair costs
~305 ns vs the 213 ns data floor.
"""

import numpy as np
import ml_dtypes

R, C, D, H = 8192, 8192, 256, 256
NCORES = 8
MSHARD = R // NCORES          # 1024 cloth rows / weight rows per core
NCT = C // 128                # 64 body-vertex 128-blocks (contraction)
NDT = D // 128                # 2 d-tiles
NHT = H // 128                # 2 h-tiles
NCB = NCT // 4                # 16 weight slabs (4 c-blocks each)

BF16 = ml_dtypes.bfloat16

_CACHE = {}


def _build_nc(reps=1, loop_iters=1):
    """Build + compile the SPMD Bass program (same program for all cores).

    reps > 1 statically repeats the whole body; loop_iters > 1 wraps the body
    in a hardware For_i loop. Both are used only by the timing harness to
    isolate per-execution device time by wall-clock slope.
    """
    import concourse.bass as bass
    import concourse.bacc as bacc
    import concourse.tile as tile
    from concourse import mybir

    fp32 = mybir.dt.float32
    bf16 = mybir.dt.bfloat16

    nc = bacc.Bacc("TRN2", target_bir_lowering=False, debug=False,
                   num_devices=NCORES)

    # DRAM I/O (per-core shapes)
    eye_d = nc.dram_tensor("eye", [128, 128], bf16,
                           kind="ExternalInput").ap()
    wt_d = nc.dram_tensor("wt", [128, NDT * H], bf16,
                          kind="ExternalInput").ap()
    b2_d = nc.dram_tensor("b2", [128, NHT], fp32, kind="ExternalInput").ap()
    nct_d = nc.dram_tensor("nct", [128, NDT * MSHARD], bf16,
                           kind="ExternalInput").ap()
    nbto_d = nc.dram_tensor("nbto", [128, NDT * MSHARD], bf16,
                            kind="ExternalInput").ap()
    nbc_d = nc.dram_tensor("nbc", [128, NCT * D], bf16,
                           kind="ExternalInput").ap()
    wpe_d = nc.dram_tensor("wpe", [NCB, 128, 8 * 512], bf16,
                           kind="ExternalInput").ap()
    top_d = nc.dram_tensor("topt_out", [NHT, 128, MSHARD], bf16,
                           kind="ExternalOutput").ap()
    x0b_d = nc.dram_tensor("x0bt_out", [NHT, 128, MSHARD], bf16,
                           kind="ExternalOutput").ap()
    # tiny completion-marker output: timing fetches only this (4 KB) to
    # await NEFF completion without a multi-MB device->host transfer
    tick_d = nc.dram_tensor("tick", [128, 8], fp32,
                            kind="ExternalOutput").ap()

    def body(tc, const_pool, wpe_pool, psY_pool, psT_pool, out_pool):
        tick_sb = out_pool.tile([128, 8], fp32, tag="tick")
        nc.vector.memset(tick_sb[:, :], 0.0)
        nc.sync.dma_start(out=tick_d[:, :], in_=tick_sb[:, :])
        eye_sb = const_pool.tile([128, 128], bf16)
        wt_sb = const_pool.tile([128, NDT * H], bf16)
        nbto_sb = const_pool.tile([128, NDT * MSHARD], bf16)
        nct_sb = const_pool.tile([128, NDT * MSHARD], bf16)
        b2_sb = const_pool.tile([128, NHT], fp32)
        nbc_sb = const_pool.tile([128, NCT * D], bf16)
        yt_bf = const_pool.tile([128, NDT * MSHARD], bf16)

        # ---- ACT-ring input DMAs (scalar-engine HWDGE): notes/w/b ----
        nc.scalar.dma_start(out=eye_sb[:, :], in_=eye_d[:, :])
        nc.scalar.dma_start(out=nct_sb[:, :], in_=nct_d[:, :])
        nc.scalar.dma_start(out=wt_sb[:, :], in_=wt_d[:, :])
        nc.scalar.dma_start(out=nbto_sb[:, :], in_=nbto_d[:, :])
        nc.scalar.dma_start(out=b2_sb[:, :], in_=b2_d[:, :])
        for p in range(8):
            nc.scalar.dma_start(out=nbc_sb[:, p * 2048:(p + 1) * 2048],
                                in_=nbc_d[:, p * 2048:(p + 1) * 2048])

        # ---- persistent PSUM tiles: 4 banks YT + 4 banks topT ----
        psY = [psY_pool.tile([128, 512], fp32, name=f"psY{g}", tag=f"psY{g}")
               for g in range(NDT * 2)]          # psY[dt*2 + mc]
        psT = [psT_pool.tile([128, 512], fp32, name=f"psT{g}", tag=f"psT{g}")
               for g in range(NHT * 2)]          # psT[ht*2 + mc]

        # ---- seed psY with NclT: top = (Ncl + W@Nb) @ w, so the cloth
        # notes fold into the [d,m] accumulators exactly via an identity
        # matmul (psY[dt,mc] = I^T @ NcT-slice), replacing the 8-MM X0cT
        # projection with 4 MMs and freeing the psT banks until the tail.
        for dt in range(NDT):
            for mc in range(2):
                nc.tensor.matmul(
                    psY[dt * 2 + mc][:, :],
                    lhsT=eye_sb[:, :],
                    rhs=nct_sb[:, dt * MSHARD + mc * 512:
                               dt * MSHARD + (mc + 1) * 512],
                    start=True, stop=False,
                )

        # ---- early PE: x0bT_own = w^T @ NbT_own (borrows psT banks) ----
        for ht in range(NHT):
            for cc in range(2):
                ps = psT[ht * 2 + cc]
                for dt in range(NDT):
                    nc.tensor.matmul(
                        ps[:, :],
                        lhsT=wt_sb[:, dt * H + ht * 128:
                                   dt * H + (ht + 1) * 128],
                        rhs=nbto_sb[:, dt * MSHARD + cc * 512:
                                    dt * MSHARD + (cc + 1) * 512],
                        start=(dt == 0), stop=(dt == NDT - 1),
                    )
                o = out_pool.tile([128, 512], bf16, tag="x0bout")
                nc.scalar.copy(out=o[:, :], in_=ps[:, :])
                nc.scalar.dma_start(
                    out=x0b_d[ht, :, cc * 512:(cc + 1) * 512], in_=o[:, :])
        # ---- main stream: YT[d,m] += nbc[c,d]^T @ W^T[c,m] ----
        for cb in range(NCB):
            wh = wpe_pool.tile([128, 8 * 512], bf16)
            nc.sync.dma_start(out=wh[:, :], in_=wpe_d[cb])
            for mc in range(2):
                for j in range(4):
                    ct = cb * 4 + j
                    for dt in range(NDT):
                        nc.tensor.matmul(
                            psY[dt * 2 + mc][:, :],
                            lhsT=nbc_sb[:, ct * D + dt * 128:
                                        ct * D + (dt + 1) * 128],
                            rhs=wh[:, mc * 2048 + j * 512:
                                    mc * 2048 + (j + 1) * 512],
                            start=False, stop=(ct == NCT - 1),
                        )
                # mc tail, staggered: as soon as this mc's accumulation is
                # closed, project through w and push the output out.
                if cb == NCB - 1:
                    for dt in range(NDT):
                        nc.vector.tensor_copy(
                            out=yt_bf[:, dt * MSHARD + mc * 512:
                                      dt * MSHARD + (mc + 1) * 512],
                            in_=psY[dt * 2 + mc][:, :])
                    for ht in range(NHT):
                        for dt in range(NDT):
                            nc.tensor.matmul(
                                psT[ht * 2 + mc][:, :],
                                lhsT=wt_sb[:, dt * H + ht * 128:
                                           dt * H + (ht + 1) * 128],
                                rhs=yt_bf[:, dt * MSHARD + mc * 512:
                                          dt * MSHARD + (mc + 1) * 512],
                                start=(dt == 0), stop=(dt == NDT - 1),
                            )
                        o = out_pool.tile([128, 512], bf16, tag="topout")
                        nc.scalar.activation(
                            o[:, :], psT[ht * 2 + mc][:, :],
                            mybir.ActivationFunctionType.Relu,
                            bias=b2_sb[:, ht:ht + 1])
                        nc.scalar.dma_start(
                            out=top_d[ht, :, mc * 512:(mc + 1) * 512],
                            in_=o[:, :])

    with tile.TileContext(nc) as tc:
        with (
            tc.tile_pool(name="const", bufs=1) as const_pool,
            tc.tile_pool(name="wpe", bufs=3) as wpe_pool,
            tc.tile_pool(name="psY", bufs=1, space="PSUM") as psY_pool,
            tc.tile_pool(name="psT", bufs=1, space="PSUM") as psT_pool,
            tc.tile_pool(name="outs", bufs=4) as out_pool,
        ):
            pools = (const_pool, wpe_pool, psY_pool, psT_pool, out_pool)
            if loop_iters > 1:
                with tc.For_i(0, loop_iters, 1,
                              hint_engines=(mybir.EngineType.PE,)):
                    body(tc, *pools)
            else:
                for _rep in range(reps):
                    body(tc, *pools)

    nc.compile()
    return nc


def _get_nc(reps=1, loop_iters=1):
    key = ("nc", reps, loop_iters)
    if key not in _CACHE:
        _CACHE[key] = _build_nc(reps, loop_iters)
    return _CACHE[key]


def _pack_inputs(notes, weight, w, b):
    """Host-side shard + transpose + bf16 cast into per-core in_maps."""
    nb = np.ascontiguousarray(notes[R:]).astype(BF16)      # (C, D)
    ncl = np.ascontiguousarray(notes[:R]).astype(BF16)     # (R, D)
    wq = w.astype(BF16)                                    # (D, H)

    eye = np.ascontiguousarray(np.eye(128, dtype=np.float32).astype(BF16))
    wt = np.ascontiguousarray(
        wq.reshape(NDT, 128, H).transpose(1, 0, 2).reshape(128, NDT * H))
    b2 = np.ascontiguousarray(b.reshape(NHT, 128).T)       # (128, NHT) f32
    # body notes as [c-part, ct*D + d] blocks (shared by all cores)
    nbc = np.ascontiguousarray(
        nb.reshape(NCT, 128, D).transpose(1, 0, 2).reshape(128, NCT * D))

    def dT(x):  # (1024, 256) -> (128, NDT*1024) [d-part, dt*M + m]
        return np.ascontiguousarray(
            x.T.reshape(NDT, 128, MSHARD).transpose(1, 0, 2)
            .reshape(128, NDT * MSHARD))

    in_maps = []
    for k in range(NCORES):
        nct = dT(ncl[k * MSHARD:(k + 1) * MSHARD])
        nbto = dT(nb[k * MSHARD:(k + 1) * MSHARD])
        wk = weight[k * MSHARD:(k + 1) * MSHARD].astype(BF16)  # (1024, 8192)
        # wpe[cb, mc, p, j*512 + ml] = wk[mc*512 + ml, (4cb+j)*128 + p]
        wpe = np.ascontiguousarray(
            wk.reshape(2, 512, NCB, 4, 128).transpose(2, 4, 0, 3, 1)
            .reshape(NCB, 128, 8 * 512))
        in_maps.append({
            "eye": eye, "wt": wt, "b2": b2, "nct": nct, "nbto": nbto,
            "nbc": nbc, "wpe": wpe,
        })
    return in_maps


def kernel(notes, weight, w, b):
    from concourse.bass_utils import run_bass_kernel_spmd

    notes = np.asarray(notes, dtype=np.float32)
    weight = np.asarray(weight, dtype=np.float32)
    w = np.asarray(w, dtype=np.float32)
    b = np.asarray(b, dtype=np.float32)

    nc = _get_nc()
    in_maps = _pack_inputs(notes, weight, w, b)
    res = run_bass_kernel_spmd(nc, in_maps, core_ids=list(range(NCORES)),
                               trace=False)

    out = np.empty((R + 2 * C, H), dtype=np.float32)
    for k in range(NCORES):
        r = res.results[k]
        out[k * MSHARD:(k + 1) * MSHARD] = \
            r["topt_out"].reshape(H, MSHARD).T.astype(np.float32)
        out[R + C + k * MSHARD:R + C + (k + 1) * MSHARD] = \
            r["x0bt_out"].reshape(H, MSHARD).T.astype(np.float32)
    out[R:R + C] = np.maximum(b, 0.0)[None, :]
    return out
